# revision 1
# baseline (speedup 1.0000x reference)
"""Trainium2 Bass kernel for nn_Encoder_85899345920647 (scatter_memory).

reference semantics:
    proj = relu(emb @ W + b) * mask            # [B, N, 32]
    scatter-add proj onto [B, H*W, 32] grid at flat loc indices
    out = concat([spatial_info, grid transposed to [B, 32, H, W]], axis=1)

Strategy (8 cores, data-parallel over B, 4 batches/core):
  - Host pre-transposes embeddings, precomputes scatter row indices, packs
    small operands into one const tensor.
  - Device: TensorE projection; is_equal selection-matrix matmul makes all
    duplicate-index rows carry the identical full sum, so colliding
    indirect-DMA row writes are benign; indirect scatter into pre-zeroed
    per-batch DRAM maps (ExternalOutput buffers are pre-zeroed by the
    runner). Map row v = (32*j + pos%32)*760 + pos//32 makes the readback
    fully contiguous per partition and a DVE 32x32 stream-transpose of the
    readback tile directly yields the channel-first output plane.
  - spatial_info channels are a DRAM->DRAM passthrough on the scalar
    engine's HWDGE ring so they never block the critical small loads on
    the sync ring (HWDGE rings are FIFO per engine).
"""

import sys

if "/opt/trn_rl_repo" not in sys.path:
    sys.path.insert(0, "/opt/trn_rl_repo")

import numpy as np

from concourse import bass, mybir
import concourse.tile as tile
from concourse.bass_utils import run_bass_kernel_spmd


F32 = mybir.dt.float32
I32 = mybir.dt.int32
F32R = mybir.dt.float32r

B, N, D_IN, D_SC = 32, 512, 256, 32
C_SP, H, W = 48, 152, 160
HW = H * W  # 24320
NCORES = 8
BPC = B // NCORES  # 4 batches per core
NBLK = N // 128  # 4 entity blocks per batch
RTOT = HW // 32  # 760 rows of 32 positions per partition-row group
NQ = 8  # densify pipeline stages
RQ = RTOT // NQ  # 190 rows per stage
PQ = RQ * 32  # 6080 positions per stage

# fconst column layout
FC_IDXP = 0  # 16 cols: scatter row idx f32, col k = j*NBLK+nb
FC_MASK = 16  # 16 cols: entity mask, same packing
FC_IDXB = 32  # 2048 cols: row idx broadcast, col j*N+n
FC_WPRJ = FC_IDXB + BPC * N  # 64 cols: W_proj [128, 2*32]
FC_BPRJ = FC_WPRJ + 2 * D_SC  # 32 cols: b_proj on row 0
FC_TOT = FC_BPRJ + D_SC  # 2176

# knobs poked by test.py
TRACE = False
LAST_EXEC_NS = None
LAST_RESULTS = None


def _build_program():
    nc = bass.Bass()

    embT = nc.dram_tensor("embT", [BPC, D_IN, N], F32, kind="ExternalInput")
    spatial = nc.dram_tensor("spatial", [BPC, C_SP, HW], F32, kind="ExternalInput")
    fconst = nc.dram_tensor("fconst", [128, FC_TOT], F32, kind="ExternalInput")
    scidx = nc.dram_tensor("scidx", [128, BPC * NBLK], I32, kind="ExternalInput")

    # split outputs: spatial passthrough and scatter plane live in separate
    # tensors so Tile never WAW-serializes their writers (host concatenates)
    out_sp = nc.dram_tensor("out_sp", [BPC, C_SP, HW], F32, kind="ExternalOutput")
    out_sc = nc.dram_tensor("out_sc", [BPC, D_SC, HW], F32, kind="ExternalOutput")
    # scatter map, pre-zeroed (ExternalOutput); row (32j + pos%32, pos//32)
    # so readback stages are single fully-contiguous DMAs (_unchain_scatters
    # removes Tile's conservative WAW chain between the 16 scatters)
    smap = nc.dram_tensor("smap", [128, RTOT, D_SC], F32, kind="ExternalOutput")

    with tile.TileContext(nc) as tc:
        with (
            tc.tile_pool(name="const", bufs=1) as cp,
            tc.tile_pool(name="work", bufs=2) as wp,
            tc.tile_pool(name="rbp", bufs=2) as rbp,
            tc.tile_pool(name="plane", bufs=2) as plp,
            tc.tile_pool(name="pp", bufs=2, space="PSUM") as pp,
            tc.tile_pool(name="pc", bufs=2, space="PSUM") as pc,
        ):
            ones1 = cp.tile([1, 128], F32)
            nc.vector.memset(ones1[:], 1.0)

            # small loads first on the sync HWDGE ring
            input_loads = []
            fc = cp.tile([128, FC_TOT], F32)
            input_loads.append(nc.sync.dma_start(out=fc[:], in_=fconst[:]))
            scidx_t = cp.tile([128, BPC * NBLK], I32)
            input_loads.append(nc.sync.dma_start(out=scidx_t[:], in_=scidx[:]))
            ets = []
            for j in range(BPC):
                et = wp.tile([128, 2, N], F32, tag="et", bufs=4)
                for kb in range(2):
                    input_loads.append(
                        nc.sync.dma_start(
                            out=et[:, kb, :],
                            in_=embT[j, kb * 128 : (kb + 1) * 128, :],
                        )
                    )
                ets.append(et)

            # bias broadcast [128, 32] built once via a K=1 matmul
            bb_ps = pc.tile([128, D_SC], F32, tag="bb")
            nc.tensor.matmul(
                out=bb_ps[:],
                lhsT=ones1[:],
                rhs=fc[0:1, FC_BPRJ : FC_BPRJ + D_SC],
                start=True,
                stop=True,
            )
            bb = cp.tile([128, D_SC], F32)
            nc.vector.tensor_copy(out=bb[:], in_=bb_ps[:])

            # per-batch: project, fix duplicates (host permuted all
            # duplicate-involved entities into tile 0), scatter
            for j in range(BPC):
                et = ets[j]
                proj_ps = pp.tile([128, NBLK, D_SC], F32)
                for nb in range(NBLK):
                    for kb in range(2):
                        nc.tensor.matmul(
                            out=proj_ps[:, nb, :],
                            lhsT=et[:, kb, nb * 128 : (nb + 1) * 128],
                            rhs=fc[
                                :, FC_WPRJ + kb * D_SC : FC_WPRJ + (kb + 1) * D_SC
                            ],
                            start=(kb == 0),
                            stop=(kb == 1),
                        )

                praw = wp.tile([128, NBLK, D_SC], F32, tag="praw")
                proj_sb = wp.tile([128, NBLK, D_SC], F32, tag="proj", bufs=4)
                for nb in range(NBLK):
                    k = j * NBLK + nb
                    nc.vector.tensor_tensor(
                        out=praw[:, nb, :],
                        in0=proj_ps[:, nb, :],
                        in1=bb[:],
                        op=mybir.AluOpType.add,
                    )
                    nc.scalar.activation(
                        out=proj_sb[:, nb, :],
                        in_=praw[:, nb, :],
                        func=mybir.ActivationFunctionType.Relu,
                        scale=fc[:, FC_MASK + k : FC_MASK + k + 1],
                    )

                # tile-0 selection matrix (all duplicate groups live here):
                # sm[p, n] = (idx0[p] == idx0[n]); comb = sm @ proj0 gives
                # every duplicate row the identical full sum
                sm = wp.tile([128, 128], F32, tag="sm", bufs=4)
                nc.vector.tensor_tensor(
                    out=sm[:],
                    in0=fc[
                        :, FC_IDXP + j * NBLK : FC_IDXP + j * NBLK + 1
                    ].to_broadcast([128, 128]),
                    in1=fc[:, FC_IDXB + j * N : FC_IDXB + j * N + 128],
                    op=mybir.AluOpType.is_equal,
                )
                comb_ps = pc.tile([128, D_SC], F32, tag="comb_ps")
                nc.tensor.matmul(
                    out=comb_ps[:],
                    lhsT=sm[:],
                    rhs=proj_sb[:, 0, :],
                    start=True,
                    stop=True,
                )
                comb_sb = wp.tile([128, D_SC], F32, tag="comb", bufs=4)
                nc.vector.tensor_copy(out=comb_sb[:], in_=comb_ps[:])

                for nb in range(NBLK):
                    k = j * NBLK + nb
                    nc.gpsimd.indirect_dma_start(
                        out=smap[:].flatten_outer_dims(),  # [128*RTOT, 32]
                        out_offset=bass.IndirectOffsetOnAxis(
                            ap=scidx_t[:, k : k + 1], axis=0
                        ),
                        in_=comb_sb[:] if nb == 0 else proj_sb[:, nb, :],
                        in_offset=None,
                    )

            # densify pipeline: contiguous readback stages, one DVE 32x32
            # block transpose each, one DMA out per stage
            for qt in range(NQ):
                r0 = qt * RQ
                rb = rbp.tile([128, RQ * D_SC], F32, tag="rb")
                nc.sync.dma_start(out=rb[:], in_=smap[:, r0 : r0 + RQ, :])
                plane = plp.tile([128, PQ], F32, tag="plane")
                nc.vector.transpose(out=plane[:], in_=rb[:])
                nc.sync.dma_start(
                    out=out_sc[:, :, qt * PQ : (qt + 1) * PQ],
                    in_=plane[:],
                )

            # spatial passthrough, on the scalar HWDGE ring (background).
            # Delayed behind the small input loads so the few MB that gate
            # all compute aren't starved by these fat transfers at t=0.
            import bass_rust as _br

            for j in range(BPC):
                h = nc.scalar.dma_start(out=out_sp[j], in_=spatial[j])
                del h  # no artificial delay: spatial bounds the tail, start it asap

    return nc


def _unchain_scatters(nc):
    """The per-batch indirect scatters write byte-identical data at any
    colliding rows, so their mutual WAW order is irrelevant. Tile chains
    them conservatively (whole-tensor writes); strip the DMASW waits from
    the scatter instructions and instead put the full set of final-value
    lane waits on the first smap readback (Sync executes in order, so
    later readbacks are covered).

    comb tiles use bufs=4 so no WAR-reuse depends transitively on the
    stripped chain; all other waits are cumulative-count semantics and
    remain valid under reordered scatter completion."""
    import bass_rust

    lane_totals = {}
    readbacks = []
    scatters = []
    for func in nc.m.functions:
        for blk in func.blocks:
            for inst in blk.instructions:
                if str(inst.opcode) != "DMACopy":
                    continue
                if getattr(inst, "queue", None) == "qPoolDynamic":
                    scatters.append(inst)
                    si = inst.sync_info
                    for u in si.on_update or []:
                        if u.ant_name.startswith("DMASW"):
                            lane_totals[u.ant_name] = (
                                lane_totals.get(u.ant_name, 0) + u.update_value
                            )
                else:
                    try:
                        ins_refs = [getattr(a, "memref", "") or "" for a in inst.ins]
                    except Exception:
                        ins_refs = []
                    if any(r.startswith("smap") for r in ins_refs):
                        readbacks.append(inst)
    if not scatters or not readbacks:
        return
    sample_wait = None
    for inst in scatters:
        si = inst.sync_info
        waits = list(si.on_wait or [])
        kept = [w for w in waits if not w.ant_name.startswith("DMASW")]
        dropped = [w for w in waits if w.ant_name.startswith("DMASW")]
        if dropped and sample_wait is None:
            sample_wait = dropped[0]
        si.on_wait = kept
    # first readback in program order gets waits for every lane's final count
    first = readbacks[0]
    si = first.sync_info
    waits = [w for w in (si.on_wait or []) if not w.ant_name.startswith("DMASW")]
    for lane, total in sorted(lane_totals.items()):
        w = bass_rust.SyncWait(
            sync_type="semaphore",
            id=next(
                x.id
                for inst2 in scatters
                for x in (inst2.sync_info.on_update or [])
                if x.ant_name == lane
            ),
            ant_name=lane,
            wait_mode="sem-ge-imm",
            wait_value=total,
            wait_reg=None,
        )
        waits.append(w)
    si.on_wait = waits


def _delay_spatial(nc):
    """Make the first spatial DRAM->DRAM copy wait for the small input
    loads: otherwise the fat spatial transfers saturate HBM from t=0 and
    the few MB of embeddings/consts that gate ALL compute crawl in at
    ~80 GB/s fair-share. Waits are absolute lane counts; only the input
    loads touch the DMAHW lanes this early, so summing their updates per
    lane is the correct wait target."""
    import bass_rust

    input_refs = {"embT", "fconst", "scidx"}
    lane_vals = {}
    lane_ids = {}
    first_spatial = None
    for func in nc.m.functions:
        for blk in func.blocks:
            for inst in blk.instructions:
                if str(inst.opcode) != "DMACopy":
                    continue
                try:
                    ins_refs = [getattr(a, "memref", "") or "" for a in inst.ins]
                except Exception:
                    ins_refs = []
                if any(r in input_refs for r in ins_refs):
                    for u in inst.sync_info.on_update or []:
                        if u.ant_name.startswith("DMAHW"):
                            lane_vals[u.ant_name] = (
                                lane_vals.get(u.ant_name, 0) + u.update_value
                            )
                            lane_ids[u.ant_name] = u.id
                elif first_spatial is None and any(r == "spatial" for r in ins_refs):
                    first_spatial = inst
    if first_spatial is None or not lane_vals:
        return
    # standalone events only: a wait placed on the DMA itself can end up
    # referencing a lane the DMA's own completion increments
    new_waits = [
        bass_rust.SyncWait(
            sync_type="semaphore",
            id=lane_ids[lane],
            ant_name=lane,
            wait_mode="sem-ge-imm",
            wait_value=val,
            wait_reg=None,
        )
        for lane, val in sorted(lane_vals.items())
    ]
    for func in nc.m.functions:
        for blk in func.blocks:
            il = blk.instructions
            try:
                idx = next(
                    i for i, inst in enumerate(il) if inst.name == first_spatial.name
                )
            except StopIteration:
                continue
            evs = []
            for ci in range(0, len(new_waits), 2):
                ev = bass_rust.InstEventSemaphore(name=f"spdelay-{ci}")
                ev.engine = first_spatial.engine
                ev.sync_info = bass_rust.SyncInfo(
                    on_wait=list(new_waits[ci : ci + 2]), on_update=[]
                )
                evs.append(ev)
            blk.instructions = il[:idx] + evs + il[idx:]
            return


def _legalize_waits(nc):
    """Split semaphore waits exceeding per-instruction ISA capacity into
    InstEventSemaphore instructions on the same engine (walrus's TRN2
    lowering holds only one sync wait per instruction; events hold two)."""
    import bass_rust

    caps = {}
    default_cap = 1
    ev_cap = 2
    counter = [0]
    for func in nc.m.functions:
        for blk in func.blocks:
            out = []
            for inst in blk.instructions:
                si = inst.sync_info
                waits = list(si.on_wait) if si is not None and si.on_wait else []
                cap = caps.get(str(inst.opcode), default_cap)
                if len(waits) > cap:
                    extra = waits[cap:]
                    for ci in range(0, len(extra), ev_cap):
                        ev = bass_rust.InstEventSemaphore(name=f"evsplit-{counter[0]}")
                        counter[0] += 1
                        ev.engine = inst.engine
                        ev.sync_info = bass_rust.SyncInfo(
                            on_wait=list(extra[ci : ci + ev_cap]), on_update=[]
                        )
                        out.append(ev)
                    si.on_wait = waits[:cap]
                out.append(inst)
            blk.instructions = out


_PROGRAM = None


def _get_program():
    global _PROGRAM
    if _PROGRAM is None:
        nc = _build_program()
        nc.finalize()
        _unchain_scatters(nc)
        _legalize_waits(nc)
        _PROGRAM = nc
    return _PROGRAM


def _pack_core_inputs(core, spatial_info, embT_all, entity_mask, v_all, W_proj, b_proj):
    j0 = core * BPC
    vf = v_all[j0 : j0 + BPC].astype(np.float32)  # [BPC, N]
    vi = v_all[j0 : j0 + BPC].astype(np.int32)
    mask = np.asarray(entity_mask[j0 : j0 + BPC], dtype=np.float32)

    def pack16(a):  # [BPC, N] -> [128, BPC*NBLK], col k = j*NBLK + nb
        return a.reshape(BPC, NBLK, 128).transpose(2, 0, 1).reshape(128, BPC * NBLK)

    fconst = np.zeros((128, FC_TOT), dtype=np.float32)
    fconst[:, FC_IDXP : FC_IDXP + 16] = pack16(vf)
    fconst[:, FC_MASK : FC_MASK + 16] = pack16(mask)
    fconst[:, FC_IDXB : FC_IDXB + BPC * N] = np.broadcast_to(
        vf.reshape(1, BPC * N), (128, BPC * N)
    )
    fconst[:, FC_WPRJ : FC_WPRJ + 2 * D_SC] = np.concatenate(
        [W_proj[:128], W_proj[128:]], axis=1
    )
    fconst[0, FC_BPRJ : FC_BPRJ + D_SC] = b_proj

    return {
        "embT": np.ascontiguousarray(embT_all[j0 : j0 + BPC]),
        "spatial": np.ascontiguousarray(
            np.asarray(spatial_info[j0 : j0 + BPC], dtype=np.float32).reshape(
                BPC, C_SP, HW
            )
        ),
        "fconst": fconst,
        "scidx": np.ascontiguousarray(pack16(vi)),
    }


def kernel(spatial_info, entity_embeddings, entity_mask, locations, W_proj, b_proj):
    global LAST_EXEC_NS, LAST_RESULTS
    spatial_info = np.asarray(spatial_info, dtype=np.float32)
    entity_embeddings = np.asarray(entity_embeddings, dtype=np.float32)
    entity_mask = np.asarray(entity_mask, dtype=np.float32)
    locations = np.asarray(locations)
    W_proj = np.asarray(W_proj, dtype=np.float32)
    b_proj = np.asarray(b_proj, dtype=np.float32)

    # host-side index math (tiny): flat position then map row. Partition
    # 32j + pos%32, per-partition row pos//32: after the DVE 32x32 block
    # transpose, value (j,c,pos) lands at plane[32j+c, pos].
    y = np.clip(locations[..., 0], 0, H - 1).astype(np.int64)
    x = np.clip(locations[..., 1], 0, W - 1).astype(np.int64)
    pos = y * W + x  # [B, N]
    v_all = (32 * ((np.arange(B) % BPC)[:, None]) + pos % 32) * RTOT + pos // 32

    embT_all = np.ascontiguousarray(
        entity_embeddings.transpose(0, 2, 1)
    )  # [B, D_IN, N]

    # permute every batch so all duplicate-involved entities sit in tile 0
    # (entities 0..127): tiles 1-3 then have globally unique rows and can
    # scatter raw proj; only tile 0 needs the selection-matrix sum.
    entity_mask = np.array(entity_mask, dtype=np.float32)
    embT_all = np.array(embT_all)
    v_all = np.array(v_all)
    for b in range(B):
        _, inv, cnt = np.unique(v_all[b], return_inverse=True, return_counts=True)
        dup = cnt[inv] >= 2
        ndup = int(dup.sum())
        assert ndup <= 128, f"batch {b}: {ndup} duplicate-involved entities > 128"
        order = np.argsort(~dup, kind="stable")
        v_all[b] = v_all[b][order]
        entity_mask[b] = entity_mask[b][order]
        embT_all[b] = embT_all[b][:, order]

    nc = _get_program()
    in_maps = [
        _pack_core_inputs(
            core, spatial_info, embT_all, entity_mask, v_all, W_proj, b_proj
        )
        for core in range(NCORES)
    ]
    res = run_bass_kernel_spmd(nc, in_maps, list(range(NCORES)), trace=TRACE)
    LAST_EXEC_NS = res.exec_time_ns
    LAST_RESULTS = res

    full = np.empty((B, C_SP + D_SC, H, W), dtype=np.float32)
    for core in range(NCORES):
        r = res.results[core]
        sl = slice(core * BPC, (core + 1) * BPC)
        full[sl, :C_SP] = r["out_sp"].reshape(BPC, C_SP, H, W)
        full[sl, C_SP:] = r["out_sc"].reshape(BPC, D_SC, H, W)
    return full



# revision 10
# speedup vs baseline: 1.8145x; 1.8145x over previous
"""Trainium2 Bass kernel for nn_Encoder_85899345920647 (scatter_memory).

reference semantics:
    proj = relu(emb @ W + b) * mask            # [B, N, 32]
    scatter-add proj onto [B, H*W, 32] grid at flat loc indices
    out = concat([spatial_info, grid transposed to [B, 32, H, W]], axis=1)

Strategy (8 cores, data-parallel over B, 4 batches/core), v2b:
  - All device staging in bf16 (host casts in/out; correctness gate is
    rel_err < 2e-2, bf16 staging lands ~1.7e-3).
  - Channel-major projection: one block-diagonal stacked-K matmul chain
    (K = 4 batches x 256 = 8 k-tiles) fills PSUM [128, W] where partition
    = 32*batch + channel, column = entity slot. relu+bias via one scalar
    activation, entity mask folded into a maskT multiply on DVE.
  - Densify IN SBUF via gpsimd local_scatter (~1.7us per 1520-position
    chunk): dst[:]=0 then dst[:, idx]=data with per-partition indices.
    Host groups entity slots by 1520-position chunk (64 slots each).
    No DRAM scatter, no readback, no transposes.
  - Duplicate positions: the group keep sits in the first 16 slots of its
    chunk; 2nd..4th occurrences live in shadow regions at fixed column
    offsets, folded in with 3 strided DVE adds before the scatters.
  - spatial passthrough: bf16 DRAM->DRAM on the sync ring AFTER the
    input loads (HWDGE FIFO = free prioritization); out_sc chunk writes
    go on the scalar ring so they never queue behind spatial.
"""

import sys

if "/opt/trn_rl_repo" not in sys.path:
    sys.path.insert(0, "/opt/trn_rl_repo")

import numpy as np
import ml_dtypes

from concourse import bass, mybir, library_config
import concourse.tile as tile
from concourse.bass_utils import run_bass_kernel_spmd
from concourse.library_overlay import lower_extended_insts

F32 = mybir.dt.float32
BF16 = mybir.dt.bfloat16
I16 = mybir.dt.int16

B, N, D_IN, D_SC = 32, 512, 256, 32
C_SP, H, W = 48, 152, 160
HW = H * W  # 24320
NCORES = 8
BPC = B // NCORES  # 4 batches per core
NKT = 2 * BPC  # 8 k-tiles of 128 (stacked K = BPC * D_IN)

NCHUNK = 16            # dense chunks per batch
CHUNK = HW // NCHUNK   # 1520 positions per chunk
SLOTS = 64             # slot columns per chunk
NDUP = 16              # dup-keep slots at the front of each chunk
MAIN = NCHUNK * SLOTS  # 1024 main cols
SH1 = MAIN             # shadow r1: [1024, 1280), 16 cols per chunk
SH2 = MAIN + 256       # shadow r2: [1280, 1536)
SH3 = MAIN + 512       # shadow r3: [1536, 1792)
WCOLS = MAIN + 768     # 1792 columns
CHUNKS = tuple(
    (c0, min(c0 + 512, WCOLS)) for c0 in range(0, WCOLS, 512)
)  # PSUM-bank (512 fp32) aligned N chunks

# knobs poked by test.py
TRACE = False
LAST_EXEC_NS = None
LAST_RESULTS = None
DEBUG_DUMP = False


def _build_program():
    nc = bass.Bass()

    embS = nc.dram_tensor("embS", [128, NKT, WCOLS], BF16, kind="ExternalInput")
    wblk = nc.dram_tensor("wblk", [128, NKT, 128], BF16, kind="ExternalInput")
    maskT = nc.dram_tensor("maskT", [128, WCOLS], BF16, kind="ExternalInput")
    sidx = nc.dram_tensor("sidx", [128, NCHUNK * SLOTS], I16, kind="ExternalInput")
    bcol = nc.dram_tensor("bcol", [128, 1], F32, kind="ExternalInput")
    spat = nc.dram_tensor("spat", [BPC, C_SP, HW], BF16, kind="ExternalInput")

    out_sp = nc.dram_tensor("out_sp", [BPC, C_SP, HW], BF16, kind="ExternalOutput")
    out_sc = nc.dram_tensor("out_sc", [BPC, D_SC, HW], BF16, kind="ExternalOutput")
    if DEBUG_DUMP:
        dbg_projM = nc.dram_tensor("dbg_projM", [128, WCOLS], BF16,
                                   kind="ExternalOutput")

    with tile.TileContext(nc) as tc:
        with (
            tc.tile_pool(name="const", bufs=1) as cp,
            tc.tile_pool(name="plane", bufs=4) as plp,
            tc.tile_pool(name="pp", bufs=1, space="PSUM") as pp,
        ):
            nc.gpsimd.load_library(library_config.local_scatter)

            # input loads on the sync ring, highest priority first
            wblk_t = cp.tile([128, NKT, 128], BF16)
            nc.sync.dma_start(out=wblk_t[:], in_=wblk[:])
            bcol_t = cp.tile([128, 1], F32)
            nc.sync.dma_start(out=bcol_t[:], in_=bcol[:])
            et = cp.tile([128, NKT, WCOLS], BF16)
            for t in range(NKT):
                nc.sync.dma_start(out=et[:, t, :], in_=embS[:, t, :])
            sidx_t = cp.tile([128, NCHUNK * SLOTS], I16)
            nc.sync.dma_start(out=sidx_t[:], in_=sidx[:])
            maskT_t = cp.tile([128, WCOLS], BF16)
            nc.sync.dma_start(out=maskT_t[:], in_=maskT[:])

            # spatial passthrough queued on the SAME sync ring: HWDGE FIFO
            # keeps it behind the loads above without any event hacks
            for j in range(BPC):
                nc.sync.dma_start(out=out_sp[j], in_=spat[j])

            # projection: t-outer so PE starts as soon as k-tile 0 lands
            psum = pp.tile([128, WCOLS], F32)
            for t in range(NKT):
                for (c0, c1) in CHUNKS:
                    nc.tensor.matmul(
                        out=psum[:, c0:c1],
                        lhsT=wblk_t[:, t, :],
                        rhs=et[:, t, c0:c1],
                        start=(t == 0),
                        stop=(t == NKT - 1),
                    )

            projT = cp.tile([128, WCOLS], BF16)
            nc.scalar.activation(
                out=projT[:], in_=psum[:],
                func=mybir.ActivationFunctionType.Relu,
                bias=bcol_t[:], scale=1.0,
            )
            projM = cp.tile([128, WCOLS], BF16)
            nc.vector.tensor_tensor(
                out=projM[:], in0=projT[:], in1=maskT_t[:],
                op=mybir.AluOpType.mult,
            )
            # fold duplicate extras (shadow regions) into the dup-keep slots:
            # main[:, 64q : 64q+16] += shadow_r[:, 16q : 16q+16] for all q at
            # once via strided views
            dkv = projM[:, 0:MAIN].rearrange("p (q s) -> p q s", s=SLOTS)[
                :, :, 0:NDUP
            ]
            for sh in (SH1, SH2, SH3):
                shv = projM[:, sh : sh + 256].rearrange(
                    "p (q s) -> p q s", s=NDUP
                )
                nc.vector.tensor_tensor(
                    out=dkv, in0=dkv, in1=shv, op=mybir.AluOpType.add,
                )

            if DEBUG_DUMP:
                nc.sync.dma_start(out=dbg_projM[:], in_=projM[:])

            # densify: local_scatter per chunk, write out on the scalar ring
            out_flat = out_sc[:].flatten_outer_dims()  # [128, HW]
            for q in range(NCHUNK):
                plane = plp.tile([128, CHUNK], BF16, tag="plane")
                nc.gpsimd.local_scatter(
                    out_ap=plane[:],
                    data_ap=projM[:, q * SLOTS : (q + 1) * SLOTS],
                    idxs_ap=sidx_t[:, q * SLOTS : (q + 1) * SLOTS],
                    channels=128, num_elems=CHUNK, num_idxs=SLOTS,
                )
                nc.scalar.dma_start(
                    out=out_flat[:, q * CHUNK : (q + 1) * CHUNK],
                    in_=plane[:],
                )

    return nc


def _legalize_waits(nc):
    """Split semaphore waits exceeding per-instruction ISA capacity into
    InstEventSemaphore instructions on the same engine (walrus's TRN2
    lowering holds only one sync wait per instruction; events hold two)."""
    import bass_rust

    default_cap = 1
    ev_cap = 2
    counter = [0]
    for func in nc.m.functions:
        for blk in func.blocks:
            out = []
            for inst in blk.instructions:
                si = inst.sync_info
                waits = list(si.on_wait) if si is not None and si.on_wait else []
                cap = default_cap
                if len(waits) > cap:
                    extra = waits[cap:]
                    for ci in range(0, len(extra), ev_cap):
                        ev = bass_rust.InstEventSemaphore(name=f"evsplit-{counter[0]}")
                        counter[0] += 1
                        ev.engine = inst.engine
                        ev.sync_info = bass_rust.SyncInfo(
                            on_wait=list(extra[ci : ci + ev_cap]), on_update=[]
                        )
                        out.append(ev)
                    si.on_wait = waits[:cap]
                out.append(inst)
            blk.instructions = out


_PROGRAM = None


def _get_program():
    global _PROGRAM
    if _PROGRAM is None:
        nc = _build_program()
        nc.finalize()
        lower_extended_insts(nc)
        _legalize_waits(nc)
        _PROGRAM = nc
    return _PROGRAM


def _assign_slots(pos_b):
    """Per-batch slot assignment, chunk-major.

    Returns (slot_col[N], sidx_rows[NCHUNK*SLOTS]) where slot_col[n] is the
    projT column of entity n and sidx_rows[64q+i] the chunk-local position
    of chunk q's slot i (or -1 for empty slots)."""
    slot_col = np.empty(N, dtype=np.int64)
    sidx_rows = np.full(NCHUNK * SLOTS, -1, dtype=np.int16)

    chunk_of = pos_b // CHUNK
    local = pos_b % CHUNK
    for q in range(NCHUNK):
        ns = np.nonzero(chunk_of == q)[0]
        if ns.size == 0:
            continue
        upos, inv, cnt = np.unique(local[ns], return_inverse=True,
                                   return_counts=True)
        ndup = int((cnt >= 2).sum())
        if upos.size > SLOTS:
            raise AssertionError(f"chunk {q}: {upos.size} distinct > {SLOTS}")
        if ndup > NDUP:
            raise AssertionError(f"chunk {q}: {ndup} dup groups > {NDUP}")
        if cnt.max() > 4:
            raise AssertionError(f"chunk {q}: multiplicity {cnt.max()} > 4")
        # slot for each distinct position: dup groups first, then singles
        dup_order = np.argsort(~(cnt >= 2), kind="stable")  # dup groups first
        slot_of_u = np.empty(upos.size, dtype=np.int64)
        slot_of_u[dup_order] = np.arange(upos.size)
        # occurrence rank within each group (order of appearance)
        seen = {}
        for n in ns:
            u = int(np.searchsorted(upos, local[n]))
            r = seen.get(u, 0)
            seen[u] = r + 1
            s = int(slot_of_u[u])
            if r == 0:
                slot_col[n] = q * SLOTS + s
                sidx_rows[q * SLOTS + s] = local[n]
            else:
                slot_col[n] = (SH1, SH2, SH3)[r - 1] + NDUP * q + s
    return slot_col, sidx_rows


def _pack_core_inputs(core, spatial16, emb, mask, pos):
    j0 = core * BPC
    embS = np.zeros((128, NKT, WCOLS), dtype=ml_dtypes.bfloat16)
    maskT = np.zeros((128, WCOLS), dtype=ml_dtypes.bfloat16)
    sidx = np.zeros((128, NCHUNK * SLOTS), dtype=np.int16)

    for j in range(BPC):
        b = j0 + j
        slot_col, sidx_rows = _assign_slots(pos[b])
        # embeddings: embS[k, 2j+kb, col] = emb[b, n, 128*kb + k]
        eb = emb[b].astype(np.float32)  # [N, D_IN]
        for kb in range(2):
            blk = np.zeros((128, WCOLS), dtype=np.float32)
            blk[:, slot_col] = eb[:, 128 * kb : 128 * (kb + 1)].T
            embS[:, 2 * j + kb, :] = blk.astype(ml_dtypes.bfloat16)
        # mask values fold into maskT
        mrow = np.zeros(WCOLS, dtype=np.float32)
        mrow[slot_col] = mask[b]
        maskT[32 * j : 32 * (j + 1), :] = mrow.astype(ml_dtypes.bfloat16)[None, :]
        sidx[32 * j : 32 * (j + 1), :] = sidx_rows[None, :]

    return {
        "embS": embS,
        "maskT": maskT,
        "sidx": sidx,
        "spat": spatial16[j0 : j0 + BPC],
    }


def kernel(spatial_info, entity_embeddings, entity_mask, locations, W_proj, b_proj):
    global LAST_EXEC_NS, LAST_RESULTS
    spatial_info = np.asarray(spatial_info, dtype=np.float32)
    entity_embeddings = np.asarray(entity_embeddings, dtype=np.float32)
    entity_mask = np.asarray(entity_mask, dtype=np.float32)
    locations = np.asarray(locations)
    W_proj = np.asarray(W_proj, dtype=np.float32)
    b_proj = np.asarray(b_proj, dtype=np.float32)

    y = np.clip(locations[..., 0], 0, H - 1).astype(np.int64)
    x = np.clip(locations[..., 1], 0, W - 1).astype(np.int64)
    pos = y * W + x  # [B, N]

    spatial16 = np.ascontiguousarray(
        spatial_info.reshape(B, C_SP, HW)
    ).astype(ml_dtypes.bfloat16)

    # shared consts: block-diagonal weights + bias column
    wblk = np.zeros((128, NKT, 128), dtype=ml_dtypes.bfloat16)
    for j in range(BPC):
        for kb in range(2):
            wblk[:, 2 * j + kb, 32 * j : 32 * (j + 1)] = (
                W_proj[128 * kb : 128 * (kb + 1), :].astype(ml_dtypes.bfloat16)
            )
    bcol = np.tile(b_proj, BPC).reshape(128, 1).astype(np.float32)

    nc = _get_program()
    in_maps = []
    for core in range(NCORES):
        m = _pack_core_inputs(core, spatial16, entity_embeddings, entity_mask, pos)
        m["wblk"] = wblk
        m["bcol"] = bcol
        in_maps.append(m)

    res = run_bass_kernel_spmd(nc, in_maps, list(range(NCORES)), trace=TRACE)
    LAST_EXEC_NS = res.exec_time_ns
    LAST_RESULTS = res

    full = np.empty((B, C_SP + D_SC, H, W), dtype=np.float32)
    for core in range(NCORES):
        r = res.results[core]
        sl = slice(core * BPC, (core + 1) * BPC)
        full[sl, :C_SP] = np.asarray(r["out_sp"]).astype(np.float32).reshape(
            BPC, C_SP, H, W
        )
        full[sl, C_SP:] = np.asarray(r["out_sc"]).astype(np.float32).reshape(
            BPC, D_SC, H, W
        )
    return full


# revision 11
# speedup vs baseline: 2.3792x; 1.3112x over previous
"""Trainium2 Bass kernel for nn_Encoder_85899345920647 (scatter_memory).

reference semantics:
    proj = relu(emb @ W + b) * mask            # [B, N, 32]
    scatter-add proj onto [B, H*W, 32] grid at flat loc indices
    out = concat([spatial_info, grid transposed to [B, 32, H, W]], axis=1)

Strategy (8 cores, data-parallel over B, 4 batches/core), v3:
  - All device staging in bf16 (host casts in/out; correctness gate is
    rel_err < 2e-2, bf16 staging lands ~1.7e-3).
  - Channel-major projection: block-diagonal stacked-K matmul chain
    (K = 4 batches x 256 = 8 k-tiles) fills PSUM [128, W] where partition
    = 32*batch + channel, column = entity slot. Chunk-outer over three
    512-col PSUM tiles so relu/mask/scatters pipeline behind the PE.
  - Densify IN SBUF via gpsimd local_scatter (~2us per 1520-position
    chunk): dst[:]=0 then dst[:, idx]=data with per-partition indices.
    Host groups entity slots by 1520-position chunk (64 slots each).
    No DRAM scatter, no readback, no transposes.
  - Duplicate positions: the group keep sits in the first 8 slots of its
    chunk; 2nd..4th occurrences live in shadow regions (columns 0..384,
    ahead of the mains) folded in with strided DVE adds per group.
  - spatial passthrough: bf16 DRAM->DRAM on the sync ring AFTER the
    input loads (HWDGE FIFO = free prioritization); out_sc chunk writes
    go on the scalar ring so they never queue behind spatial.
"""

import sys

if "/opt/trn_rl_repo" not in sys.path:
    sys.path.insert(0, "/opt/trn_rl_repo")

import numpy as np
import ml_dtypes

from concourse import bass, mybir, library_config
import concourse.tile as tile
from concourse.bass_utils import run_bass_kernel_spmd
from concourse.library_overlay import lower_extended_insts

F32 = mybir.dt.float32
BF16 = mybir.dt.bfloat16
I16 = mybir.dt.int16

B, N, D_IN, D_SC = 32, 512, 256, 32
C_SP, H, W = 48, 152, 160
HW = H * W  # 24320
NCORES = 8
BPC = B // NCORES  # 4 batches per core
NKT = 2 * BPC  # 8 k-tiles of 128 (stacked K = BPC * D_IN)

NCHUNK = 16            # dense chunks per batch
CHUNK = HW // NCHUNK   # 1520 positions per chunk
SLOTS = 64             # slot columns per chunk
NDUP = 8               # dup-keep slots at the front of each chunk
SH1 = 0                # shadow r1: [0, 128), 8 cols per chunk
SH2 = 128              # shadow r2: [128, 256)
SH3 = 256              # shadow r3: [256, 384)
MAIN0 = 384            # mains: [384, 1408), chunk q at 384 + 64q
WCOLS = MAIN0 + NCHUNK * SLOTS  # 1408 columns
# PSUM column chunks and which dense chunks' mains they hold
PCS = ((0, 512, 0, 2), (512, 1024, 2, 10), (1024, WCOLS, 10, 16))

# knobs poked by test.py
TRACE = False
LAST_EXEC_NS = None
LAST_RESULTS = None
DEBUG_DUMP = False


def _build_program():
    nc = bass.Bass()

    embS = nc.dram_tensor("embS", [128, NKT, WCOLS], BF16, kind="ExternalInput")
    wblk = nc.dram_tensor("wblk", [128, NKT, 128], BF16, kind="ExternalInput")
    maskT = nc.dram_tensor("maskT", [128, WCOLS], BF16, kind="ExternalInput")
    sidx = nc.dram_tensor("sidx", [128, NCHUNK * SLOTS], I16, kind="ExternalInput")
    bcol = nc.dram_tensor("bcol", [128, 1], F32, kind="ExternalInput")
    spat = nc.dram_tensor("spat", [BPC, C_SP, HW], BF16, kind="ExternalInput")

    out_sp = nc.dram_tensor("out_sp", [BPC, C_SP, HW], BF16, kind="ExternalOutput")
    out_sc = nc.dram_tensor("out_sc", [BPC, D_SC, HW], BF16, kind="ExternalOutput")
    if DEBUG_DUMP:
        dbg_projM = nc.dram_tensor("dbg_projM", [128, WCOLS], BF16,
                                   kind="ExternalOutput")

    with tile.TileContext(nc) as tc:
        with (
            tc.tile_pool(name="const", bufs=1) as cp,
            tc.tile_pool(name="proj", bufs=4) as prp,
            tc.tile_pool(name="plane", bufs=8) as plp,
            tc.tile_pool(name="pp", bufs=4, space="PSUM") as pp,
        ):
            nc.gpsimd.load_library(library_config.local_scatter)

            # input loads on the sync ring, highest priority first
            et = cp.tile([128, NKT, WCOLS], BF16)
            for t in range(NKT):
                nc.sync.dma_start(out=et[:, t, :], in_=embS[:, t, :])
            wblk_t = cp.tile([128, NKT, 128], BF16)
            nc.sync.dma_start(out=wblk_t[:], in_=wblk[:])
            bcol_t = cp.tile([128, 1], F32)
            nc.sync.dma_start(out=bcol_t[:], in_=bcol[:])
            sidx_t = cp.tile([128, NCHUNK * SLOTS], I16)
            nc.sync.dma_start(out=sidx_t[:], in_=sidx[:])
            maskT_t = cp.tile([128, WCOLS], BF16)
            nc.sync.dma_start(out=maskT_t[:], in_=maskT[:])

            # spatial passthrough queued on the SAME sync ring: HWDGE FIFO
            # keeps it behind the loads above without any event hacks
            for j in range(BPC):
                nc.sync.dma_start(out=out_sp[j], in_=spat[j])

            # projection, chunk-outer: each PSUM column chunk finishes its
            # 8-k-tile accumulation, then relu (scalar) + mask (vector)
            pms = []
            for (c0, c1, q0, q1) in PCS:
                nco = c1 - c0
                ps = pp.tile([128, nco], F32, tag="ps")
                for t in range(NKT):
                    nc.tensor.matmul(
                        out=ps[:],
                        lhsT=wblk_t[:, t, :],
                        rhs=et[:, t, c0:c1],
                        start=(t == 0),
                        stop=(t == NKT - 1),
                    )
                pt = prp.tile([128, nco], BF16, tag="pt")
                nc.scalar.activation(
                    out=pt[:], in_=ps[:],
                    func=mybir.ActivationFunctionType.Relu,
                    bias=bcol_t[:], scale=1.0,
                )
                pm = prp.tile([128, nco], BF16, tag="pm")
                nc.vector.tensor_tensor(
                    out=pm[:], in0=pt[:], in1=maskT_t[:, c0:c1],
                    op=mybir.AluOpType.mult,
                )
                pms.append(pm)

            # shadow views (all live in pms[0], cols 0..384)
            shv = [
                pms[0][:, r : r + 128].rearrange("p (q s) -> p q s", s=NDUP)
                for r in (SH1, SH2, SH3)
            ]

            def main_ap(q):
                """[128, 64] view of dense-chunk q's main slot columns."""
                for i, (c0, c1, q0, q1) in enumerate(PCS):
                    if q0 <= q < q1:
                        off = MAIN0 + 64 * q - c0
                        return pms[i][:, off : off + 64]
                raise AssertionError

            out_flat = out_sc[:].flatten_outer_dims()  # [128, HW]
            for (c0, c1, q0, q1) in PCS:
                # fold duplicate extras into the dup-keep slots of q0..q1
                i = PCS.index((c0, c1, q0, q1))
                mv = pms[i][
                    :, MAIN0 + 64 * q0 - c0 : MAIN0 + 64 * q1 - c0
                ].rearrange("p (q s) -> p q s", s=SLOTS)[:, :, 0:NDUP]
                for r in range(3):
                    nc.vector.tensor_tensor(
                        out=mv, in0=mv, in1=shv[r][:, q0:q1, :],
                        op=mybir.AluOpType.add,
                    )
                # densify + write out
                for q in range(q0, q1):
                    plane = plp.tile([128, CHUNK], BF16, tag="plane")
                    nc.gpsimd.local_scatter(
                        out_ap=plane[:],
                        data_ap=main_ap(q),
                        idxs_ap=sidx_t[:, q * SLOTS : (q + 1) * SLOTS],
                        channels=128, num_elems=CHUNK, num_idxs=SLOTS,
                    )
                    nc.scalar.dma_start(
                        out=out_flat[:, q * CHUNK : (q + 1) * CHUNK],
                        in_=plane[:],
                    )

            if DEBUG_DUMP:
                for i, (c0, c1, q0, q1) in enumerate(PCS):
                    nc.sync.dma_start(out=dbg_projM[:, c0:c1], in_=pms[i][:])

    return nc


def _legalize_waits(nc):
    """Split semaphore waits exceeding per-instruction ISA capacity into
    InstEventSemaphore instructions on the same engine (walrus's TRN2
    lowering holds only one sync wait per instruction; events hold two)."""
    import bass_rust

    default_cap = 1
    ev_cap = 2
    counter = [0]
    for func in nc.m.functions:
        for blk in func.blocks:
            out = []
            for inst in blk.instructions:
                si = inst.sync_info
                waits = list(si.on_wait) if si is not None and si.on_wait else []
                cap = default_cap
                if len(waits) > cap:
                    extra = waits[cap:]
                    for ci in range(0, len(extra), ev_cap):
                        ev = bass_rust.InstEventSemaphore(name=f"evsplit-{counter[0]}")
                        counter[0] += 1
                        ev.engine = inst.engine
                        ev.sync_info = bass_rust.SyncInfo(
                            on_wait=list(extra[ci : ci + ev_cap]), on_update=[]
                        )
                        out.append(ev)
                    si.on_wait = waits[:cap]
                out.append(inst)
            counter[0] = counter[0]
            blk.instructions = out


_PROGRAM = None


def _get_program():
    global _PROGRAM
    if _PROGRAM is None:
        nc = _build_program()
        nc.finalize()
        lower_extended_insts(nc)
        _legalize_waits(nc)
        _PROGRAM = nc
    return _PROGRAM


def _assign_slots(pos_b):
    """Per-batch slot assignment, chunk-major.

    Returns (slot_col[N], sidx_rows[NCHUNK*SLOTS]) where slot_col[n] is the
    projM column of entity n and sidx_rows[64q+i] the chunk-local position
    of chunk q's slot i (or -1 for empty slots)."""
    slot_col = np.empty(N, dtype=np.int64)
    sidx_rows = np.full(NCHUNK * SLOTS, -1, dtype=np.int16)

    chunk_of = pos_b // CHUNK
    local = pos_b % CHUNK
    for q in range(NCHUNK):
        ns = np.nonzero(chunk_of == q)[0]
        if ns.size == 0:
            continue
        upos, cnt = np.unique(local[ns], return_counts=True)
        ndup = int((cnt >= 2).sum())
        if upos.size > SLOTS:
            raise AssertionError(f"chunk {q}: {upos.size} distinct > {SLOTS}")
        if ndup > NDUP:
            raise AssertionError(f"chunk {q}: {ndup} dup groups > {NDUP}")
        if cnt.max() > 4:
            raise AssertionError(f"chunk {q}: multiplicity {cnt.max()} > 4")
        # slot for each distinct position: dup groups first, then singles
        dup_order = np.argsort(~(cnt >= 2), kind="stable")
        slot_of_u = np.empty(upos.size, dtype=np.int64)
        slot_of_u[dup_order] = np.arange(upos.size)
        seen = {}
        for n in ns:
            u = int(np.searchsorted(upos, local[n]))
            r = seen.get(u, 0)
            seen[u] = r + 1
            s = int(slot_of_u[u])
            if r == 0:
                slot_col[n] = MAIN0 + q * SLOTS + s
                sidx_rows[q * SLOTS + s] = local[n]
            else:
                slot_col[n] = (SH1, SH2, SH3)[r - 1] + NDUP * q + s
    return slot_col, sidx_rows


def _pack_core_inputs(core, spatial16, emb, mask, pos):
    j0 = core * BPC
    embS = np.zeros((128, NKT, WCOLS), dtype=ml_dtypes.bfloat16)
    maskT = np.zeros((128, WCOLS), dtype=ml_dtypes.bfloat16)
    sidx = np.zeros((128, NCHUNK * SLOTS), dtype=np.int16)

    for j in range(BPC):
        b = j0 + j
        slot_col, sidx_rows = _assign_slots(pos[b])
        # embeddings: embS[k, 2j+kb, col] = emb[b, n, 128*kb + k]
        eb = emb[b].astype(np.float32)  # [N, D_IN]
        for kb in range(2):
            blk = np.zeros((128, WCOLS), dtype=np.float32)
            blk[:, slot_col] = eb[:, 128 * kb : 128 * (kb + 1)].T
            embS[:, 2 * j + kb, :] = blk.astype(ml_dtypes.bfloat16)
        # mask values fold into maskT
        mrow = np.zeros(WCOLS, dtype=np.float32)
        mrow[slot_col] = mask[b]
        maskT[32 * j : 32 * (j + 1), :] = mrow.astype(ml_dtypes.bfloat16)[None, :]
        sidx[32 * j : 32 * (j + 1), :] = sidx_rows[None, :]

    return {
        "embS": embS,
        "maskT": maskT,
        "sidx": sidx,
        "spat": spatial16[j0 : j0 + BPC],
    }


def kernel(spatial_info, entity_embeddings, entity_mask, locations, W_proj, b_proj):
    global LAST_EXEC_NS, LAST_RESULTS
    spatial_info = np.asarray(spatial_info, dtype=np.float32)
    entity_embeddings = np.asarray(entity_embeddings, dtype=np.float32)
    entity_mask = np.asarray(entity_mask, dtype=np.float32)
    locations = np.asarray(locations)
    W_proj = np.asarray(W_proj, dtype=np.float32)
    b_proj = np.asarray(b_proj, dtype=np.float32)

    y = np.clip(locations[..., 0], 0, H - 1).astype(np.int64)
    x = np.clip(locations[..., 1], 0, W - 1).astype(np.int64)
    pos = y * W + x  # [B, N]

    spatial16 = np.ascontiguousarray(
        spatial_info.reshape(B, C_SP, HW)
    ).astype(ml_dtypes.bfloat16)

    # shared consts: block-diagonal weights + bias column
    wblk = np.zeros((128, NKT, 128), dtype=ml_dtypes.bfloat16)
    for j in range(BPC):
        for kb in range(2):
            wblk[:, 2 * j + kb, 32 * j : 32 * (j + 1)] = (
                W_proj[128 * kb : 128 * (kb + 1), :].astype(ml_dtypes.bfloat16)
            )
    bcol = np.tile(b_proj, BPC).reshape(128, 1).astype(np.float32)

    nc = _get_program()
    in_maps = []
    for core in range(NCORES):
        m = _pack_core_inputs(core, spatial16, entity_embeddings, entity_mask, pos)
        m["wblk"] = wblk
        m["bcol"] = bcol
        in_maps.append(m)

    res = run_bass_kernel_spmd(nc, in_maps, list(range(NCORES)), trace=TRACE)
    LAST_EXEC_NS = res.exec_time_ns
    LAST_RESULTS = res

    full = np.empty((B, C_SP + D_SC, H, W), dtype=np.float32)
    for core in range(NCORES):
        r = res.results[core]
        sl = slice(core * BPC, (core + 1) * BPC)
        full[sl, :C_SP] = np.asarray(r["out_sp"]).astype(np.float32).reshape(
            BPC, C_SP, H, W
        )
        full[sl, C_SP:] = np.asarray(r["out_sc"]).astype(np.float32).reshape(
            BPC, D_SC, H, W
        )
    return full
